# revision 30
# baseline (speedup 1.0000x reference)
"""Trainium2 Bass kernel for nn_ContextPooling.

Reference computation (per batch b):
  x = feats[b].T                                  # [D, L]
  x1 = lrelu(LN(conv1d(x, w1, b1)))               # [2D, L]
  x2 = lrelu(LN(conv1d(x1, w2, b2)))              # [2D, L]
  x3 = lrelu(LN(conv1d(x2, w3, b3)))              # [2, L]
  s, w = softmax(x3[0]), softmax(x3[1])           # [L]
  invd[i] = 1/(eps + 2*(R*L*s[i])^2)
  G[i,j] = exp(-(i-j)^2 * invd[i])                # (ret/rowmax simplifies to this)
  out[b,i,:] = sum_j G[i,j] * w[j] * feats[b,j,:]

Sharding: data-parallel over batch, 1 batch per NeuronCore (B=8, 8 cores).
Convs run as shifted accumulated matmuls in float32r (fp32-identical numerics
on HW, 4x faster than plain fp32); conv padding is handled with clipped
edge matmuls (PSUM has_written semantics make partial-width first-writes
safe). The Gaussian is built on-device per 128x128 tile as
band_const @ diag(invd_chunk) followed by ACT exp. All ACT functions map
into one LUT table set (parametric_relu / exp / ln) to avoid table reloads.
"""
import numpy as np
from contextlib import ExitStack

import concourse.bacc as bacc
import concourse.tile as tile
import concourse.mybir as mybir
from concourse.bass_utils import run_bass_kernel_spmd

F32 = mybir.dt.float32
F32R = mybir.dt.float32r
AF = mybir.ActivationFunctionType
ALU = mybir.AluOpType
AX = mybir.AxisListType

B, L, D = 8, 1024, 1024
C1 = 2 * D          # 2048
P = 128
NT1 = D // P        # 8  input-channel tiles, layer 1
NT2 = C1 // P       # 16 channel tiles, layers 2/3
NCH = L // 512      # 2  N-chunks per row
EPS_LN = 1e-5
EPS_STD = 1e-5
SLOPE = 0.01
RL = 0.1 * L        # 102.4
C_STD = 2.0 * RL * RL  # 20971.52
LP = L + 2          # padded row length for conv inputs

_CACHE = {}

_ACT_SET = "natural_log_exp_and_others"   # contains parametric_relu, exp, ln


def _pin_act_tables():
    """Restrict the ACT table-set chooser to one set containing every
    function this kernel uses, so no mid-kernel LUT reloads are emitted.
    Indices must stay aligned with act_info.json, so other sets are
    emptied rather than removed."""
    if _CACHE.get("act_pinned"):
        return
    orig = bacc.get_activation_tables

    def pinned(arch):
        t = orig(arch)
        return {name: (s if name == _ACT_SET else set()) for name, s in t.items()}

    pinned.__wrapped__ = orig
    bacc.get_activation_tables = pinned
    _CACHE["act_pinned"] = True


def _clip(c, k):
    """Conv shift k in {0,1,2}: rhs offset and psum column range."""
    o = 512 * c + k - 1
    lo, hi = max(0, o), min(L, o + 512)
    return lo, hi, lo - o


def _ln_scalars(nc, pool, psum_small, mv, nparts, ones_col, tag):
    """Per-partition (mean, var) [nparts,2] -> [nparts,2] tile of
    (rstd, -mean*rstd) broadcast to all partitions."""
    stats2 = pool.tile([nparts, 2], F32, tag=f"st2_{tag}")
    nc.vector.tensor_copy(stats2[:, 0:1], mv[:, 0:1])
    tmp = pool.tile([nparts, 1], F32, tag=f"tmp_{tag}")
    nc.vector.tensor_mul(tmp[:], mv[:, 0:1], mv[:, 0:1])
    nc.vector.tensor_add(stats2[:, 1:2], mv[:, 1:2], tmp[:])
    ps = psum_small.tile([P, 2], F32, tag="small")
    nc.tensor.matmul(ps[0:1, :], ones_col[:nparts, 0:1], stats2[:], start=True, stop=True)
    sc = pool.tile([1, 8], F32, tag=f"sc_{tag}")
    epst = pool.tile([1, 1], F32, tag=f"eps_{tag}")
    nc.vector.memset(epst[:], EPS_LN)
    rb = pool.tile([1, 2], F32, tag=f"rb_{tag}")
    # sc[0:2] = (mean, E[x^2])
    nc.vector.tensor_scalar_mul(sc[:, 0:2], ps[0:1, 0:2], 1.0 / nparts)
    nc.vector.tensor_mul(sc[:, 2:3], sc[:, 0:1], sc[:, 0:1])       # mean^2
    nc.vector.tensor_sub(sc[:, 3:4], sc[:, 1:2], sc[:, 2:3])       # var
    # rstd = exp(-0.5*ln(var+eps)); ln/exp stay in the same ACT table set
    nc.scalar.activation(sc[:, 4:5], sc[:, 3:4], AF.Ln, bias=epst[:], scale=1.0)
    nc.scalar.activation(rb[:, 0:1], sc[:, 4:5], AF.Exp, scale=-0.5)
    # rb[1] = -mean*rstd
    nc.vector.tensor_scalar(out=rb[:, 1:2], in0=sc[:, 0:1], scalar1=rb[:, 0:1],
                            scalar2=-1.0, op0=ALU.mult, op1=ALU.mult)
    ps2 = psum_small.tile([P, 2], F32, tag="small")
    nc.tensor.matmul(ps2[:nparts, :], ones_col[0:1, :nparts], rb[:], start=True, stop=True)
    sb = pool.tile([nparts, 2], F32, tag=f"sb_{tag}")
    nc.vector.tensor_copy(sb[:], ps2[:nparts, :])
    return sb


def build_program():
    _pin_act_tables()
    nc = bacc.Bacc("TRN2", target_bir_lowering=False, debug=False)

    featsT_d = nc.dram_tensor("featsT", [D, L], F32R, kind="ExternalInput").ap()
    feats_d = nc.dram_tensor("feats", [L, D], F32, kind="ExternalInput").ap()
    w1s_d = nc.dram_tensor("w1s", [NT2, P, NT1, 3, P], F32R, kind="ExternalInput").ap()
    w2s_d = nc.dram_tensor("w2s", [NT2, 2, P, NT2 // 2, 3, P], F32R, kind="ExternalInput").ap()
    w3s_d = nc.dram_tensor("w3s", [NT2, P, 3, 2], F32R, kind="ExternalInput").ap()
    b1_d = nc.dram_tensor("b1", [C1], F32, kind="ExternalInput").ap()
    b2_d = nc.dram_tensor("b2", [C1], F32, kind="ExternalInput").ap()
    b3_d = nc.dram_tensor("b3", [2, 1], F32, kind="ExternalInput").ap()
    band_d = nc.dram_tensor("band", [P, 15 * P], F32R, kind="ExternalInput").ap()
    ident_d = nc.dram_tensor("ident", [P, P], F32, kind="ExternalInput").ap()
    out_d = nc.dram_tensor("out", [L, D], F32, kind="ExternalOutput").ap()

    with tile.TileContext(nc) as tc, ExitStack() as ctx:
        arena = ctx.enter_context(tc.tile_pool(name="arena", bufs=1))
        wpool = ctx.enter_context(tc.tile_pool(name="wpool", bufs=3))
        small = ctx.enter_context(tc.tile_pool(name="small", bufs=1))
        outp = ctx.enter_context(tc.tile_pool(name="outp", bufs=2))
        lrtmp_pool = ctx.enter_context(tc.tile_pool(name="lrtmp", bufs=2))

        def _apply_lrelu(u, x, sb, n_act=10):
            """u[:, t, 1:L+1] = lrelu(x[:, t]*rstd + nbias); tiles < n_act on
            ACT (LUT Prelu), the rest on DVE (exact affine + max) in parallel."""
            nc.scalar.activation(u[:, 0, 1:514], x[:, 0, 0:513], AF.Prelu,
                                 bias=sb[:, 1:2], scale=sb[:, 0:1], alpha=SLOPE)
            nc.scalar.activation(u[:, 0, 514:L + 1], x[:, 0, 513:L], AF.Prelu,
                                 bias=sb[:, 1:2], scale=sb[:, 0:1], alpha=SLOPE)
            for t in range(1, n_act):
                nc.scalar.activation(u[:, t, 1:L + 1], x[:, t, :], AF.Prelu,
                                     bias=sb[:, 1:2], scale=sb[:, 0:1], alpha=SLOPE)
            for t in range(n_act, NT2):
                for h in range(2):
                    tmp = lrtmp_pool.tile([P, 512], F32, tag="lrtmp")
                    nc.vector.tensor_scalar(out=tmp[:], in0=x[:, t, 512 * h:512 * (h + 1)],
                                            scalar1=sb[:, 0:1], scalar2=sb[:, 1:2],
                                            op0=ALU.mult, op1=ALU.add)
                    nc.vector.scalar_tensor_tensor(
                        out=u[:, t, 1 + 512 * h:1 + 512 * (h + 1)], in0=tmp[:],
                        scalar=SLOPE, in1=tmp[:], op0=ALU.mult, op1=ALU.max)
        psum_mm = ctx.enter_context(tc.tile_pool(name="psum_mm", bufs=3, space="PSUM"))
        psum_g = ctx.enter_context(tc.tile_pool(name="psum_g", bufs=2, space="PSUM"))
        psum_c3 = ctx.enter_context(tc.tile_pool(name="psum_c3", bufs=1, space="PSUM"))
        psum_small = ctx.enter_context(tc.tile_pool(name="psum_small", bufs=1, space="PSUM"))

        # ---------- conv1 input ----------
        # All conv-input tiles share the arena "A" slot with one padded
        # layout [P, NT2, LP]; pad columns (0 and L+1) are zeroed once here
        # and stay zero for the whole kernel (later tiles in the slot only
        # ever write the interior columns).
        # First conv1 weight slab loads first: it gates the very first matmul.
        slab0 = wpool.tile([P, NT1, 3, P], F32R, tag="slab")
        xT = arena.tile([P, NT2, LP], F32R, tag="A")
        # startup DMAs in exact first-use order
        nc.sync.dma_start(slab0[:, 0:2], w1s_d[0, :, 0:2])
        nc.sync.dma_start(xT[:, 0, 1:L + 1], featsT_d[0:P, :])
        nc.sync.dma_start(xT[:, 1, 1:L + 1], featsT_d[P:2 * P, :])
        nc.sync.dma_start(slab0[:, 2:NT1], w1s_d[0, :, 2:NT1])
        for t in range(NT2):
            nc.vector.memset(xT[:, t, 0:1].bitcast(F32), 0.0)
            nc.vector.memset(xT[:, t, L + 1:LP].bitcast(F32), 0.0)
        for t in range(2, NT1):
            nc.sync.dma_start(xT[:, t, 1:L + 1], featsT_d[t * P:(t + 1) * P, :])

        ones_col = small.tile([P, P], F32)
        nc.vector.memset(ones_col[:], 1.0)
        b1_sb = small.tile([P, NT2], F32)
        nc.sync.dma_start(b1_sb[:], b1_d.rearrange("(m p) -> p m", p=P))
        # warm the ACT table set early (off the critical path)
        warm = small.tile([1, 1], F32, tag="warm")
        nc.scalar.activation(warm[:], ones_col[0:1, 0:1], AF.Prelu, scale=1.0,
                             alpha=SLOPE)

        # ---------- conv1 ----------
        x1 = arena.tile([P, NT2, L], F32, tag="B")
        st1 = small.tile([P, 2 * NT2, 6], F32, tag="st1")
        for m in range(NT2):
            if m == 0:
                slab = slab0
            else:
                slab = wpool.tile([P, NT1, 3, P], F32R, tag="slab")
                nc.sync.dma_start(slab[:, 0:NT1 // 2], w1s_d[m, :, 0:NT1 // 2])
                nc.sync.dma_start(slab[:, NT1 // 2:], w1s_d[m, :, NT1 // 2:])
            for c in range(NCH):
                ps = psum_mm.tile([P, 512], F32, tag="mm")
                nmm = NT1 * 3
                i = 0
                for t in range(NT1):
                    for k in range(3):
                        o = 512 * c + k
                        nc.tensor.matmul(ps[:], slab[:, t, k, :],
                                         xT[:, t, o:o + 512],
                                         start=(i == 0), stop=(i == nmm - 1))
                        i += 1
                nc.vector.tensor_scalar_add(x1[:, m, 512 * c:512 * (c + 1)], ps[:],
                                            b1_sb[:, m:m + 1])
                nc.vector.bn_stats(st1[:, NCH * m + c, :], x1[:, m, 512 * c:512 * (c + 1)])

        # ---------- LN1 + lrelu ----------
        mv1 = small.tile([P, 2], F32, tag="mv1")
        nc.vector.bn_aggr(mv1[:], st1[:])
        sb1 = _ln_scalars(nc, small, psum_small, mv1, P, ones_col, "l1")
        u1 = arena.tile([P, NT2, LP], F32R, tag="A")
        _apply_lrelu(u1, x1, sb1)

        # b2 while conv2 runs
        b2_sb = small.tile([P, NT2], F32)
        nc.sync.dma_start(b2_sb[:], b2_d.rearrange("(m p) -> p m", p=P))

        # ---------- conv2 ----------
        x2 = arena.tile([P, NT2, L], F32, tag="B")
        st2 = small.tile([P, 2 * NT2, 6], F32, tag="st2")
        for m in range(NT2):
            slabs = []
            for h in range(2):
                sl = wpool.tile([P, NT2 // 2, 3, P], F32R, tag="slab")
                nc.sync.dma_start(sl[:], w2s_d[m, h])
                slabs.append(sl)
            for c in range(NCH):
                ps = psum_mm.tile([P, 512], F32, tag="mm")
                nmm = NT2 * 3
                i = 0
                for h in range(2):
                    for t in range(NT2 // 2):
                        for k in range(3):
                            o = 512 * c + k
                            nc.tensor.matmul(ps[:], slabs[h][:, t, k, :],
                                             u1[:, 8 * h + t, o:o + 512],
                                             start=(i == 0), stop=(i == nmm - 1))
                            i += 1
                nc.vector.tensor_scalar_add(x2[:, m, 512 * c:512 * (c + 1)], ps[:],
                                            b2_sb[:, m:m + 1])
                nc.vector.bn_stats(st2[:, NCH * m + c, :], x2[:, m, 512 * c:512 * (c + 1)])

        # constants needed later (DMA'd during conv2)
        w3_t = small.tile([P, NT2, 3, 2], F32R)
        nc.sync.dma_start(w3_t[:], w3s_d.rearrange("t p k c -> p t k c"))
        b3_sb = small.tile([2, 1], F32)
        nc.sync.dma_start(b3_sb[:], b3_d[:])
        band_t = small.tile([P, 15 * P], F32R)
        nc.sync.dma_start(band_t[:], band_d[:])
        ident_t = small.tile([P, P], F32)
        nc.sync.dma_start(ident_t[:], ident_d[:])

        # ---------- LN2 + lrelu ----------
        mv2 = small.tile([P, 2], F32, tag="mv2")
        nc.vector.bn_aggr(mv2[:], st2[:])
        sb2 = _ln_scalars(nc, small, psum_small, mv2, P, ones_col, "l2")
        u2 = arena.tile([P, NT2, LP], F32R, tag="A")
        _apply_lrelu(u2, x2, sb2)

        # ---------- conv3 ----------
        x3 = small.tile([2, L], F32, tag="x3")
        for c in range(NCH):
            ps3 = psum_c3.tile([2, 512], F32, tag="c3")
            nmm = NT2 * 3
            i = 0
            for t in range(NT2):
                for k in range(3):
                    o = 512 * c + k
                    nc.tensor.matmul(ps3[:], w3_t[:, t, k, :],
                                     u2[:, t, o:o + 512],
                                     start=(i == 0), stop=(i == nmm - 1))
                    i += 1
            nc.vector.tensor_scalar_add(x3[:, 512 * c:512 * (c + 1)], ps3[:], b3_sb[:, 0:1])

        # feats for the einsum rhs (DMA during conv3)
        feats8 = arena.tile([P, NT1, D], F32, tag="B")
        for jt in range(NT1):
            nc.sync.dma_start(feats8[:, jt, :], feats_d[jt * P:(jt + 1) * P, :])

        # ---------- transpose x3 -> [128, 16] then LN3/softmax in wide layout --
        psT = psum_small.tile([P, 16], F32, tag="cols")
        for j in range(NT1):
            nc.tensor.matmul(psT[:, 2 * j:2 * j + 2], x3[0:2, j * P:(j + 1) * P],
                             ident_t[0:2, 0:2], start=True, stop=True)
        x3T = small.tile([P, 16], F32, tag="x3T")
        nc.vector.tensor_copy(x3T[:], psT[:])
        st3 = small.tile([P, 6], F32, tag="st3b")
        nc.vector.bn_stats(st3[:], psT[:])
        mv3 = small.tile([P, 2], F32, tag="mv3")
        nc.vector.bn_aggr(mv3[:], st3[:])
        sb3 = _ln_scalars(nc, small, psum_small, mv3, P, ones_col, "l3")
        # lrelu + exp (softmax without max-sub: post-LN logits are ~N(0,1))
        x3n = small.tile([P, 16], F32, tag="x3n")
        nc.scalar.activation(x3n[:], x3T[:], AF.Prelu,
                             bias=sb3[:, 1:2], scale=sb3[:, 0:1], alpha=SLOPE)
        e3 = small.tile([P, 16], F32, tag="e3")
        nc.scalar.activation(e3[:], x3n[:], AF.Exp, scale=1.0)
        # per-partition partial sums over j for (s, w) -> cross-partition sums
        part = small.tile([P, 2], F32, tag="part")
        nc.vector.tensor_reduce(part[:], e3[:].rearrange("p (j c) -> p c j", c=2),
                                axis=AX.X, op=ALU.add)
        ps_sc = psum_small.tile([P, 2], F32, tag="small")
        nc.tensor.matmul(ps_sc[0:1, :], ones_col[:, 0:1], part[:], start=True, stop=True)
        # scalars: cp = C_STD / Ssum^2 ; rw = 1 / Wsum
        scc = small.tile([1, 4], F32, tag="scc")
        nc.vector.tensor_copy(scc[:, 0:2], ps_sc[0:1, 0:2])
        nc.vector.tensor_mul(scc[:, 2:3], scc[:, 0:1], scc[:, 0:1])
        nc.vector.reciprocal(scc[:, 3:4], scc[:, 2:3])
        bc2 = small.tile([1, 2], F32, tag="bc2")
        nc.vector.tensor_scalar_mul(bc2[:, 0:1], scc[:, 3:4], C_STD)
        nc.vector.reciprocal(bc2[:, 1:2], scc[:, 1:2])
        ps_b2 = psum_small.tile([P, 2], F32, tag="small")
        nc.tensor.matmul(ps_b2[:], ones_col[0:1, :], bc2[:], start=True, stop=True)
        cw = small.tile([P, 2], F32, tag="cw")
        nc.vector.tensor_copy(cw[:], ps_b2[:])
        # icol[:, it] = 1 / (eps + cp * s_exp^2)
        icol = small.tile([P, NT1], F32, tag="icol")
        nc.vector.tensor_mul(icol[:], e3[:, 0:16:2], e3[:, 0:16:2])
        nc.vector.tensor_scalar(out=icol[:], in0=icol[:], scalar1=cw[:, 0:1],
                                scalar2=EPS_STD, op0=ALU.mult, op1=ALU.add)
        nc.vector.reciprocal(icol[:], icol[:])

        # fw ([:,0]) and gT ([:,1]) share one f32r tile in the tag-A slot
        egT = arena.tile([P, 2, NT1, L], F32R, tag="A")
        fw = egT[:, 0]
        gT = egT[:, 1]
        for jt in reversed(range(NT1)):
            nc.vector.tensor_scalar(out=fw[:, jt, 0:512],
                                    in0=feats8[:, jt, 0:512],
                                    scalar1=e3[:, 2 * jt + 1:2 * jt + 2],
                                    scalar2=cw[:, 1:2], op0=ALU.mult, op1=ALU.mult)
        for jt in reversed(range(NT1)):
            nc.vector.tensor_scalar(out=fw[:, jt, 512:L],
                                    in0=feats8[:, jt, 512:L],
                                    scalar1=e3[:, 2 * jt + 1:2 * jt + 2],
                                    scalar2=cw[:, 1:2], op0=ALU.mult, op1=ALU.mult)

        # ---------- gauss tiles (batched along diagonals) + einsum ----------
        # Tiles with equal delta = it - jt share the band lhsT; the rhs packs
        # up to 4 diag(invd-chunk) blocks into one N<=512 matmul. diag blocks
        # are built on GPSIMD to keep DVE free for fw. Einsum group `it`
        # is emitted right after diagonal delta=it (its last dependency);
        # within a group jt accumulates descending so deps match diagonal
        # completion order.
        def _chunks(n):
            if n <= 4:
                return [(0, n)]
            if n <= 6:
                return [(0, n - 2), (n - 2, 2)]
            return [(0, 4), (4, n - 4)]

        def _gauss_group(delta):
            jt0 = max(0, -delta)
            nb_all = NT1 - abs(delta)
            lhsT = band_t[:, (delta + 7) * P:(delta + 8) * P]
            for (boff, nb) in _chunks(nb_all):
                dblk = wpool.tile([P, 512], F32R, tag="dblk")
                for b in range(nb):
                    it_b = jt0 + boff + b + delta
                    nc.gpsimd.tensor_scalar_mul(dblk[:, b * P:(b + 1) * P],
                                                ident_t[:], icol[:, it_b:it_b + 1])
                psg = psum_g.tile([P, 512], F32, tag="g")
                nc.tensor.matmul(psg[:, 0:nb * P], lhsT, dblk[:, 0:nb * P],
                                 start=True, stop=True)
                for b in range(nb):
                    jt_b = jt0 + boff + b
                    it_b = jt_b + delta
                    nc.scalar.activation(gT[:, jt_b, it_b * P:(it_b + 1) * P],
                                         psg[:, b * P:(b + 1) * P], AF.Exp, scale=-1.0)

        def _einsum_group(it, c, split_evac=False):
            pse = psum_mm.tile([P, 512], F32, tag="mm")
            for n, jt in enumerate(reversed(range(NT1))):
                nc.tensor.matmul(pse[:], gT[:, jt, it * P:(it + 1) * P],
                                 fw[:, jt, 512 * c:512 * (c + 1)],
                                 start=(n == 0), stop=(n == NT1 - 1))
            oev = outp.tile([P, 512], F32, tag="oev")
            if split_evac:
                for h in range(4):
                    nc.vector.tensor_copy(oev[:, 128 * h:128 * (h + 1)],
                                          pse[:, 128 * h:128 * (h + 1)])
                    nc.sync.dma_start(
                        out_d[it * P:(it + 1) * P,
                              512 * c + 128 * h:512 * c + 128 * (h + 1)],
                        oev[:, 128 * h:128 * (h + 1)])
            else:
                nc.vector.tensor_copy(oev[:], pse[:])
                nc.sync.dma_start(out_d[it * P:(it + 1) * P, 512 * c:512 * (c + 1)], oev[:])

        for delta in range(-(NT1 - 1), 1):
            _gauss_group(delta)
        for delta in range(0, NT1):
            if delta > 0:
                _gauss_group(delta)
            _einsum_group(delta, 0)
            _einsum_group(delta, 1)

    nc.compile()
    return nc


def _host_prep(w1, w2, w3, b1, b2, b3):
    """Weight relayout + constants (input-independent of feats)."""
    w1s = np.ascontiguousarray(
        w1.reshape(NT2, P, NT1, P, 3).transpose(0, 3, 2, 4, 1))  # [16,128,8,3,128]
    w2s = np.ascontiguousarray(
        w2.reshape(NT2, P, 2, NT2 // 2, P, 3).transpose(0, 2, 4, 3, 5, 1))
    w3s = np.ascontiguousarray(
        w3.reshape(2, NT2, P, 3).transpose(1, 2, 3, 0))  # [16,128,3,2]
    r = np.arange(P, dtype=np.float64)
    dblk = np.arange(15, dtype=np.float64)
    j = np.arange(P, dtype=np.float64)
    band = ((128.0 * (dblk[None, :, None] - 7) + r[:, None, None] - j[None, None, :]) ** 2)
    band = band.reshape(P, 15 * P).astype(np.float32)
    ident = np.eye(P, dtype=np.float32)
    return {
        "w1s": w1s, "w2s": w2s, "w3s": w3s,
        "b1": np.ascontiguousarray(b1), "b2": np.ascontiguousarray(b2),
        "b3": np.ascontiguousarray(b3.reshape(2, 1)),
        "band": band, "ident": ident,
    }


def kernel(feats, w1, b1, w2, b2, w3, b3):
    feats = np.asarray(feats, dtype=np.float32)
    if "nc" not in _CACHE:
        _CACHE["nc"] = build_program()
    nc = _CACHE["nc"]
    common = _host_prep(np.asarray(w1, np.float32), np.asarray(w2, np.float32),
                        np.asarray(w3, np.float32), np.asarray(b1, np.float32),
                        np.asarray(b2, np.float32), np.asarray(b3, np.float32))
    in_maps = []
    for b in range(B):
        m = dict(common)
        m["feats"] = np.ascontiguousarray(feats[b])
        m["featsT"] = np.ascontiguousarray(feats[b].T)
        in_maps.append(m)
    last_err = None
    for attempt in range(3):
        try:
            res = run_bass_kernel_spmd(nc, in_maps, list(range(B)))
            break
        except Exception as e:  # transient NRT/device failures: rebuild + retry
            last_err = e
            _CACHE.pop("nc", None)
            nc = _CACHE.setdefault("nc", build_program())
    else:
        raise last_err
    out = np.stack([res.results[b]["out"] for b in range(B)], axis=0)
    return out.astype(np.float32)
